# revision 4
# baseline (speedup 1.0000x reference)
"""Trainium2 Bass kernel for nn_CholeskyResHead_68255620268805.

Reference math (per mixture component c of C=10):
    Ks = Ls @ Ls.T ; Kt = Lt @ Lt.T            (spatial 207x207, temporal 12x12)
    M  = (Ks  (x)  Kt + sig^2 I)^-1            (via eigh + explicit kron in ref)
    quad[b,c] = r_b^T M r_b                    (r = (target-mu).reshape(b, n*t))
    ll = -0.5*n*t*log(2pi) - 0.5*quad + n*Vlog + t*Ulog + log w
    nll_loss = mean_b(-logsumexp_c ll)
    mse_loss = mean(|mu-target| * mask/mean(mask)),  mask = (unscaled != 0)
    out = 0.1*nll_loss + 0.9*mse_loss

Key identity used here: with Ks = Us Ds Us^T, Kt = Ut Dt Ut^T,
    quad[b,c] = sum_{m,j} (Us^T R_b Ut)[m,j]^2 / (Ds[m] Dt[j] + sig^2)
so the (nt x nt) kron inverse never needs to be materialized.

Distribution: data-parallel over batch (64 -> 8 per core), all 10 components
on every core; no collectives.  Host does the small eigendecompositions
(parameter prep, invariant for the quadratic form) and the final 8-way
scalar combine; the device does everything batch-sized: residuals, the two
GEMM stages, the capacitance-weighted square-reduce, quad assembly, the
log-sum-exp, and the masked-MAE partial sums.

Perf notes (measured via in-NEFF For_i loop deltas):
  - per-dma_start issue cost dominates small transfers -> all component
    parameters ship as a handful of wide coalesced DMAs, split across the
    two HWDGE rings (sync + scalar).
  - components are processed in pairs (414-wide tiles) to halve PE/ACT
    instruction counts.
"""

import numpy as np

B, N, T, C = 64, 207, 12, 10
NT = N * T
RHO = 0.1
LOG2PI = float(np.log(2.0 * np.pi))
NCORES = 8
BL = B // NCORES          # local batches per core
BT = BL * T               # 96 = (b, t) pairs per core
P0 = 128                  # first spatial chunk (partition dim)
P1 = N - P0               # 79
NPAIR = C // 2

_CACHE: dict = {}


def _declare_io(nc, f32):
    t = {}
    t["mu_t"] = nc.dram_tensor("mu_t", [N, BT], f32, kind="ExternalInput")
    t["tg_t"] = nc.dram_tensor("tg_t", [N, BT], f32, kind="ExternalInput")
    t["un_t"] = nc.dram_tensor("un_t", [N, BT], f32, kind="ExternalInput")
    # us_t[n, c*N+m] = Us_c[n, m]
    t["us_t"] = nc.dram_tensor("us_t", [N, C * N], f32, kind="ExternalInput")
    # wk_t[k, c*BT+q] = kron(I_BL, Ut_c)[k, q]
    t["wk_t"] = nc.dram_tensor("wk_t", [BT, C * BT], f32, kind="ExternalInput")
    # ic_t[q, c*N+m] = 1/(Ds_c[m]*Dt_c[j]+sig_c^2), q=(b,j)
    t["ic_t"] = nc.dram_tensor("ic_t", [BT, C * N], f32, kind="ExternalInput")
    # aux: [0:BT, 0:BL] = kron(I_BL, ones(T,1)); [:, BL:BL+1] = ones;
    #      [0:BL, BL+1:BL+1+C] = ll constants (incl. logw)
    t["aux"] = nc.dram_tensor("aux", [P0, BL + 1 + C], f32, kind="ExternalInput")
    t["out_part"] = nc.dram_tensor("out_part", [1, 3], f32, kind="ExternalOutput")
    t["quad_out"] = nc.dram_tensor("quad_out", [BL, C], f32, kind="ExternalOutput")
    return t


def _emit_body(nc, tc, io):
    import concourse.mybir as mybir

    f32 = mybir.dt.float32
    AF = mybir.ActivationFunctionType
    OP = mybir.AluOpType
    AX = mybir.AxisListType

    with (
        tc.tile_pool(name="cst", bufs=1) as cst,
        tc.tile_pool(name="ztp", bufs=3) as ztp,
        tc.tile_pool(name="sqp", bufs=3) as sqp,
        tc.tile_pool(name="scp", bufs=2) as scp,
        tc.tile_pool(name="ps_z", bufs=2, space="PSUM") as ps_z,
        tc.tile_pool(name="ps_y", bufs=2, space="PSUM") as ps_y,
        tc.tile_pool(name="ps_s", bufs=1, space="PSUM") as ps_s,
    ):
        # ---- coalesced loads; batch tensors on the sync HWDGE ring,
        # ---- component parameters on the scalar HWDGE ring ----
        mu0 = cst.tile([P0, BT], f32, tag="mu0")
        mu1 = cst.tile([P1, BT], f32, tag="mu1")
        tg0 = cst.tile([P0, BT], f32, tag="tg0")
        tg1 = cst.tile([P1, BT], f32, tag="tg1")
        un0 = cst.tile([P0, BT], f32, tag="un0")
        un1 = cst.tile([P1, BT], f32, tag="un1")
        nc.sync.dma_start(mu0[:], io["mu_t"][0:P0, :])
        nc.sync.dma_start(mu1[:], io["mu_t"][P0:N, :])
        nc.sync.dma_start(tg0[:], io["tg_t"][0:P0, :])
        nc.sync.dma_start(tg1[:], io["tg_t"][P0:N, :])
        nc.sync.dma_start(un0[:], io["un_t"][0:P0, :])
        nc.sync.dma_start(un1[:], io["un_t"][P0:N, :])
        aux = cst.tile([P0, BL + 1 + C], f32, tag="aux")
        nc.sync.dma_start(aux[:], io["aux"][:])

        us0a = cst.tile([P0, C * N], f32, tag="us0a")
        us1a = cst.tile([P1, C * N], f32, tag="us1a")
        nc.scalar.dma_start(us0a[:], io["us_t"][0:P0, :])
        nc.scalar.dma_start(us1a[:], io["us_t"][P0:N, :])
        wka = cst.tile([BT, C * BT], f32, tag="wka")
        nc.scalar.dma_start(wka[:], io["wk_t"][:])
        ica = cst.tile([BT, C * N], f32, tag="ica")
        nc.scalar.dma_start(ica[:], io["ic_t"][:])

        emt = aux[0:BT, 0:BL]
        onest = aux[:, BL : BL + 1]
        m2t = aux[0:BL, BL + 1 : BL + 1 + C]

        # ---- residuals (also the matmul lhsT) ----
        r0 = cst.tile([P0, BT], f32, tag="r0")
        r1 = cst.tile([P1, BT], f32, tag="r1")
        nc.vector.tensor_sub(r0[:], tg0[:], mu0[:])
        nc.vector.tensor_sub(r1[:], tg1[:], mu1[:])

        # ---- masked-MAE partial sums ----
        mk0 = cst.tile([P0, BT], f32, tag="mk0")
        mk1 = cst.tile([P1, BT], f32, tag="mk1")
        nc.vector.tensor_scalar(mk0[:], un0[:], 0.0, None, op0=OP.not_equal)
        nc.vector.tensor_scalar(mk1[:], un1[:], 0.0, None, op0=OP.not_equal)
        mr0 = cst.tile([P0, BT], f32, tag="mr0")
        mr1 = cst.tile([P1, BT], f32, tag="mr1")
        nc.vector.tensor_mul(mr0[:], r0[:], mk0[:])
        nc.vector.tensor_mul(mr1[:], r1[:], mk1[:])
        pt0 = cst.tile([P0, 2], f32, tag="pt0")
        pt1 = cst.tile([P1, 2], f32, tag="pt1")
        nc.vector.tensor_reduce(
            pt0[:, 0:1], mr0[:], axis=AX.X, op=OP.add, apply_absolute_value=True
        )
        nc.vector.tensor_reduce(pt0[:, 1:2], mk0[:], axis=AX.X, op=OP.add)
        nc.vector.tensor_reduce(
            pt1[:, 0:1], mr1[:], axis=AX.X, op=OP.add, apply_absolute_value=True
        )
        nc.vector.tensor_reduce(pt1[:, 1:2], mk1[:], axis=AX.X, op=OP.add)
        mae_ps = ps_s.tile([1, 2], f32, tag="mae_ps")
        nc.tensor.matmul(mae_ps[:], onest[:], pt0[:], start=True, stop=False)
        nc.tensor.matmul(mae_ps[:], onest[0:P1, :], pt1[:], start=False, stop=True)

        # ---- per-component quadratic forms, two components per tile ----
        # S[(b,j), c] = sum_m (Us_c^T R_b Ut_c)[m,j]^2 * icap_c[j,m]
        S = cst.tile([BT, C], f32, tag="S")
        for p in range(NPAIR):
            zt = ps_z.tile([BT, 2 * N], f32, tag="zt")
            for h in range(2):
                c = 2 * p + h
                cz = zt[:, h * N : (h + 1) * N]
                nc.tensor.matmul(
                    cz, r0[:], us0a[:, c * N : (c + 1) * N], start=True, stop=False
                )
                nc.tensor.matmul(
                    cz, r1[:], us1a[:, c * N : (c + 1) * N], start=False, stop=True
                )
            ztsb = ztp.tile([BT, 2 * N], f32, tag="ztsb")
            if p % 2 == 0:
                nc.scalar.copy(ztsb[:], zt[:])
            else:
                nc.vector.tensor_copy(ztsb[:], zt[:])

            yt = ps_y.tile([BT, 2 * N], f32, tag="yt")
            for h in range(2):
                c = 2 * p + h
                nc.tensor.matmul(
                    yt[:, h * N : (h + 1) * N],
                    wka[:, c * BT : (c + 1) * BT],
                    ztsb[:, h * N : (h + 1) * N],
                    start=True,
                    stop=True,
                )
            sq = sqp.tile([BT, 2 * N], f32, tag="sq")
            nc.scalar.activation(sq[:], yt[:], AF.Square)
            for h in range(2):
                c = 2 * p + h
                scr = scp.tile([BT, N], f32, tag="scr")
                # scr = (sq * 1.0) * ic ; S[:,c] = sum_m scr
                nc.vector.scalar_tensor_tensor(
                    scr[:],
                    sq[:, h * N : (h + 1) * N],
                    1.0,
                    ica[:, c * N : (c + 1) * N],
                    op0=OP.mult,
                    op1=OP.mult,
                    accum_out=S[:, c : c + 1],
                )

        # ---- quad[b, c] = sum_j S[(b,j), c] ----
        q_ps = ps_s.tile([BL, C], f32, tag="q_ps")
        nc.tensor.matmul(q_ps[:], emt[:], S[:], start=True, stop=True)
        q_sb = cst.tile([BL, C], f32, tag="q_sb")
        nc.scalar.copy(q_sb[:], q_ps[:])
        nc.sync.dma_start(io["quad_out"][:], q_sb[:])

        # ---- ll + logsumexp over components ----
        ll = cst.tile([BL, C], f32, tag="ll")
        nc.vector.scalar_tensor_tensor(
            ll[:], q_ps[:], -0.5, m2t[:], op0=OP.mult, op1=OP.add
        )
        negmx = cst.tile([BL, 1], f32, tag="negmx")
        nc.vector.tensor_reduce(negmx[:], ll[:], axis=AX.X, op=OP.max, negate=True)
        ex = cst.tile([BL, C], f32, tag="ex")
        nc.scalar.activation(ex[:], ll[:], AF.Exp, bias=negmx[:, 0:1])
        se = cst.tile([BL, 1], f32, tag="se")
        nc.vector.tensor_reduce(se[:], ex[:], axis=AX.X, op=OP.add)
        lse = cst.tile([BL, 1], f32, tag="lse")
        nc.scalar.activation(lse[:], se[:], AF.Ln)
        v = cst.tile([BL, 1], f32, tag="v")
        nc.vector.tensor_sub(v[:], lse[:], negmx[:])  # = logsumexp_c ll

        sv_ps = ps_s.tile([1, 1], f32, tag="sv_ps")
        nc.tensor.matmul(sv_ps[:], onest[0:BL, :], v[:], start=True, stop=True)

        # ---- pack per-core partials: [sum_b lse, sum |r|*mask, sum mask] ----
        part = cst.tile([1, 3], f32, tag="part")
        nc.scalar.copy(part[0:1, 0:1], sv_ps[:])
        nc.scalar.copy(part[0:1, 1:3], mae_ps[:])
        nc.sync.dma_start(io["out_part"][:], part[:])


def _build_program():
    import concourse.bacc as bacc
    import concourse.mybir as mybir
    from concourse import tile

    f32 = mybir.dt.float32
    nc = bacc.Bacc(None, target_bir_lowering=False)
    io = _declare_io(nc, f32)
    with tile.TileContext(nc) as tc:
        _emit_body(nc, tc, io)
    nc.compile()
    return nc


def _get_program():
    if "nc" not in _CACHE:
        _CACHE["nc"] = _build_program()
    return _CACHE["nc"]


def _host_prep(mu, target, unscaled_target, w, sigma, L_spatial, L_temporal):
    """Per-core input maps. Heavy lifting (eigh of the small covariance
    factors) in float64 for accuracy; everything shipped as float32."""
    f = np.float32
    mu = np.asarray(mu, dtype=f)
    target = np.asarray(target, dtype=f)
    unscaled_target = np.asarray(unscaled_target, dtype=f)
    w = np.asarray(w, dtype=f)
    Ls = np.asarray(L_spatial, dtype=np.float64)
    Lt = np.asarray(L_temporal, dtype=np.float64)

    Ks = Ls @ np.transpose(Ls, (0, 2, 1))        # (C, N, N)
    Kt = Lt @ np.transpose(Lt, (0, 2, 1))        # (C, T, T)
    Ds, Us = np.linalg.eigh(Ks)                   # (C, N), (C, N, N)
    Dt, Ut = np.linalg.eigh(Kt)                   # (C, T), (C, T, T)
    sig2 = np.asarray(sigma, dtype=np.float64) ** 2

    # icap[c, j, m] = 1 / (Ds[c, m] * Dt[c, j] + sig2[c])
    icap = 1.0 / (Dt[:, :, None] * Ds[:, None, :] + sig2[:, None, None])
    icr = np.tile(icap, (1, BL, 1)).astype(f)               # (C, BT, N)
    ic_t = np.ascontiguousarray(icr.transpose(1, 0, 2).reshape(BT, C * N))
    wblk = np.stack([np.kron(np.eye(BL), Ut[c]) for c in range(C)]).astype(f)
    wk_t = np.ascontiguousarray(wblk.transpose(1, 0, 2).reshape(BT, C * BT))
    us_t = np.ascontiguousarray(
        Us.astype(f).transpose(1, 0, 2).reshape(N, C * N)
    )

    Ulog = np.sum(np.log(np.einsum("cii->ci", Ls)), axis=1)       # spatial
    Vlog = np.sum(np.log(np.einsum("cii->ci", Lt)), axis=1)       # temporal
    logw = np.log(np.asarray(w, dtype=np.float64)[..., 0])        # (B, C)
    m2_full = (
        -0.5 * NT * LOG2PI + N * Vlog[None, :] + T * Ulog[None, :] + logw
    ).astype(f)                                                    # (B, C)

    in_maps = []
    for k in range(NCORES):
        sl = slice(k * BL, (k + 1) * BL)
        tr = lambda x: np.ascontiguousarray(
            x[sl].transpose(1, 0, 2).reshape(N, BT)
        )
        aux = np.zeros((P0, BL + 1 + C), dtype=f)
        aux[0:BT, 0:BL] = np.kron(np.eye(BL, dtype=f), np.ones((T, 1), dtype=f))
        aux[:, BL] = 1.0
        aux[0:BL, BL + 1 : BL + 1 + C] = m2_full[sl]
        in_maps.append(
            {
                "mu_t": tr(mu),
                "tg_t": tr(target),
                "un_t": tr(unscaled_target),
                "us_t": us_t,
                "wk_t": wk_t,
                "ic_t": ic_t,
                "aux": aux,
            }
        )
    return in_maps


def kernel(**inputs) -> np.ndarray:
    from concourse.bass_utils import run_bass_kernel_spmd

    nc = _get_program()
    in_maps = _host_prep(
        inputs["mu"],
        inputs["target"],
        inputs["unscaled_target"],
        inputs["w"],
        inputs["sigma"],
        inputs["L_spatial"],
        inputs["L_temporal"],
    )
    res = run_bass_kernel_spmd(nc, in_maps, list(range(NCORES))).results

    sum_lse = 0.0
    sum_abs = 0.0
    sum_msk = 0.0
    for k in range(NCORES):
        p = res[k]["out_part"]
        sum_lse += float(p[0, 0])
        sum_abs += float(p[0, 1])
        sum_msk += float(p[0, 2])
    nll_loss = -(np.float32(sum_lse) / np.float32(B))
    mse_loss = np.float32(sum_abs) / np.float32(sum_msk)
    out = np.float32(RHO) * nll_loss + np.float32(1.0 - RHO) * mse_loss
    return np.asarray(out, dtype=np.float32)


# revision 6
# speedup vs baseline: 1.3648x; 1.3648x over previous
"""Trainium2 Bass kernel for nn_CholeskyResHead_68255620268805.

Reference math (per mixture component c of C=10):
    Ks = Ls @ Ls.T ; Kt = Lt @ Lt.T            (spatial 207x207, temporal 12x12)
    M  = (Ks  (x)  Kt + sig^2 I)^-1            (via eigh + explicit kron in ref)
    quad[b,c] = r_b^T M r_b                    (r = (target-mu).reshape(b, n*t))
    ll = -0.5*n*t*log(2pi) - 0.5*quad + n*Vlog + t*Ulog + log w
    nll_loss = mean_b(-logsumexp_c ll)
    mse_loss = mean(|mu-target| * mask/mean(mask)),  mask = (unscaled != 0)
    out = 0.1*nll_loss + 0.9*mse_loss

Key identity: with Ks = Us Ds Us^T, Kt = Ut Dt Ut^T,
    quad[b,c] = sum_{m,j} (Us^T R_b Ut)[m,j]^2 / (Ds[m] Dt[j] + sig^2)
so the (nt x nt) kron inverse never needs to be materialized.  The
temporal transform is batched through a block-diagonal kron(I_8, Ut)
stationary matrix so one matmul handles 8 batches.

Distribution (HW-measured: this environment has a hard per-core DMA
bandwidth wall of ~78 GB/s, so per-core input bytes are the whole game):
a 4 component-groups x 2 batch-halves grid.  Each core gets 3 component
slots (groups (3,3,2,2), zero-padded) and 32 batches.  Residuals are
computed host-side and shipped sharded (as the sharding hint suggests);
the MAE mask ships as packed uint8.  Per-core input = 1.27 MB vs 3.3 MB
for pure batch-parallel with replicated eigenbases.  The device computes
the two GEMM stages, the capacitance-weighted square-reduce, the quad
assembly matmul, and the masked-MAE partial sums; the host does the tiny
(64,10) log-sum-exp and the final scalar combine.
"""

import numpy as np

B, N, T, C = 64, 207, 12, 10
NT = N * T
RHO = 0.1
LOG2PI = float(np.log(2.0 * np.pi))
NCORES = 8

G_B = 2                 # batch halves
G_C = 4                 # component groups
BH = B // G_B           # 32 batches per core
BTL = BH * T            # 384 (b,t) pairs per core
NQ = 4                  # batch chunks of 8 per core
BL = 8                  # batches per chunk
BT = BL * T             # 96 rows per chunk
CL = 3                  # component slots per core (padded)
P0 = 128
P1 = N - P0             # 79
COMP_GROUPS = [[0, 1, 2], [3, 4, 5], [6, 7], [8, 9]]

# in0/in1 column layout (f32 columns)
US_W = CL * N           # 621
RS_OFF = US_W           # 621
MK_OFF = RS_OFF + BTL   # 1005  (mask: BTL u8 = BTL/4 f32 cols)
AUX_OFF = MK_OFF + BTL // 4   # 1101
W0 = AUX_OFF + 16       # 1117
IC_W = CL * N           # 621
WIK = IC_W + CL * BT    # 909

_CACHE: dict = {}


def _declare_io(nc, f32):
    t = {}
    t["in0"] = nc.dram_tensor("in0", [P0, W0], f32, kind="ExternalInput")
    t["in1"] = nc.dram_tensor("in1", [P1, W0], f32, kind="ExternalInput")
    t["icwk"] = nc.dram_tensor("icwk", [BT, WIK], f32, kind="ExternalInput")
    t["oq"] = nc.dram_tensor("oq", [BL, NQ * CL + 2], f32, kind="ExternalOutput")
    return t


def _emit_body(nc, tc, io):
    import concourse.mybir as mybir

    f32 = mybir.dt.float32
    u8 = mybir.dt.uint8
    AF = mybir.ActivationFunctionType
    OP = mybir.AluOpType
    AX = mybir.AxisListType

    with (
        tc.tile_pool(name="cst", bufs=1) as cst,
        tc.tile_pool(name="ztp", bufs=3) as ztp,
        tc.tile_pool(name="sqp", bufs=3) as sqp,
        tc.tile_pool(name="scp", bufs=2) as scp,
        tc.tile_pool(name="ps_z", bufs=3, space="PSUM") as ps_z,
        tc.tile_pool(name="ps_y", bufs=2, space="PSUM") as ps_y,
        tc.tile_pool(name="ps_s", bufs=1, space="PSUM") as ps_s,
    ):
        in0t = cst.tile([P0, W0], f32, tag="in0t")
        in1t = cst.tile([P1, W0], f32, tag="in1t")
        iwt = cst.tile([BT, WIK], f32, tag="iwt")
        nc.sync.dma_start(in0t[:], io["in0"][:])
        nc.scalar.dma_start(in1t[:], io["in1"][:])
        nc.scalar.dma_start(iwt[:], io["icwk"][:])

        rs0 = in0t[:, RS_OFF : RS_OFF + BTL]
        rs1 = in1t[:, RS_OFF : RS_OFF + BTL]
        emt = in0t[0:BT, AUX_OFF : AUX_OFF + BL]
        onest = in0t[:, AUX_OFF + BL : AUX_OFF + BL + 1]

        # ---- masked-MAE partial sums ----
        mk0u = in0t[:, MK_OFF:AUX_OFF].bitcast(u8)
        mk1u = in1t[:, MK_OFF:AUX_OFF].bitcast(u8)
        mkf0 = cst.tile([P0, BTL], f32, tag="mkf0")
        mkf1 = cst.tile([P1, BTL], f32, tag="mkf1")
        nc.vector.tensor_copy(mkf0[:], mk0u)
        nc.vector.tensor_copy(mkf1[:], mk1u)
        mr0 = cst.tile([P0, BTL], f32, tag="mr0")
        mr1 = cst.tile([P1, BTL], f32, tag="mr1")
        nc.vector.tensor_mul(mr0[:], rs0, mkf0[:])
        nc.vector.tensor_mul(mr1[:], rs1, mkf1[:])
        pt0 = cst.tile([P0, 2], f32, tag="pt0")
        pt1 = cst.tile([P1, 2], f32, tag="pt1")
        nc.vector.tensor_reduce(
            pt0[:, 0:1], mr0[:], axis=AX.X, op=OP.add, apply_absolute_value=True
        )
        nc.vector.tensor_reduce(pt0[:, 1:2], mkf0[:], axis=AX.X, op=OP.add)
        nc.vector.tensor_reduce(
            pt1[:, 0:1], mr1[:], axis=AX.X, op=OP.add, apply_absolute_value=True
        )
        nc.vector.tensor_reduce(pt1[:, 1:2], mkf1[:], axis=AX.X, op=OP.add)
        mae_ps = ps_s.tile([1, 2], f32, tag="mae_ps")
        nc.tensor.matmul(mae_ps[:], onest, pt0[:], start=True, stop=False)
        nc.tensor.matmul(mae_ps[:], onest[0:P1, :], pt1[:], start=False, stop=True)

        # ---- per-(component-slot, batch-chunk) quadratic forms ----
        # S[(b,j), q*CL+cl] = sum_m (Us^T R_b Ut)[m,j]^2 * icap[j,m]
        S = cst.tile([BT, NQ * CL], f32, tag="S")
        for cl in range(CL):
            us0 = in0t[:, cl * N : (cl + 1) * N]
            us1 = in1t[:, cl * N : (cl + 1) * N]
            ict = iwt[:, cl * N : (cl + 1) * N]
            wkt = iwt[:, IC_W + cl * BT : IC_W + (cl + 1) * BT]
            for q in range(NQ):
                zt = ps_z.tile([BT, N], f32, tag="zt")
                lhs0 = rs0[:, q * BT : (q + 1) * BT]
                lhs1 = rs1[:, q * BT : (q + 1) * BT]
                nc.tensor.matmul(zt[:], lhs0, us0, start=True, stop=False)
                nc.tensor.matmul(zt[:], lhs1, us1, start=False, stop=True)
                ztsb = ztp.tile([BT, N], f32, tag="ztsb")
                if (cl * NQ + q) % 2 == 0:
                    nc.scalar.copy(ztsb[:], zt[:])
                else:
                    nc.vector.tensor_copy(ztsb[:], zt[:])

                yt = ps_y.tile([BT, N], f32, tag="yt")
                nc.tensor.matmul(yt[:], wkt, ztsb[:], start=True, stop=True)
                sq = sqp.tile([BT, N], f32, tag="sq")
                nc.scalar.activation(sq[:], yt[:], AF.Square)
                scr = scp.tile([BT, N], f32, tag="scr")
                # scr = (sq * 1.0) * ic ; S[:,col] = sum_m scr
                nc.vector.scalar_tensor_tensor(
                    scr[:],
                    sq[:],
                    1.0,
                    ict,
                    op0=OP.mult,
                    op1=OP.mult,
                    accum_out=S[:, q * CL + cl : q * CL + cl + 1],
                )

        # ---- quad[b, (q,cl)] = sum_j S[(b,j), (q,cl)] ----
        q_ps = ps_s.tile([BL, NQ * CL], f32, tag="q_ps")
        nc.tensor.matmul(q_ps[:], emt, S[:], start=True, stop=True)

        # ---- pack outputs: quad (8, 12) + [mae_abs, mae_cnt] on row 0 ----
        ot = cst.tile([BL, NQ * CL + 2], f32, tag="ot")
        nc.scalar.copy(ot[:, 0 : NQ * CL], q_ps[:])
        nc.vector.tensor_scalar(
            ot[:, NQ * CL : NQ * CL + 2], q_ps[:, 0:2], 0.0, None, op0=OP.mult
        )
        nc.scalar.copy(ot[0:1, NQ * CL : NQ * CL + 2], mae_ps[:])
        nc.sync.dma_start(io["oq"][:], ot[:])


def _build_program():
    import concourse.bacc as bacc
    import concourse.mybir as mybir
    from concourse import tile

    f32 = mybir.dt.float32
    nc = bacc.Bacc(None, target_bir_lowering=False)
    io = _declare_io(nc, f32)
    with tile.TileContext(nc) as tc:
        _emit_body(nc, tc, io)
    nc.compile()
    return nc


def _get_program():
    if "nc" not in _CACHE:
        _CACHE["nc"] = _build_program()
    return _CACHE["nc"]


def _host_prep(mu, target, unscaled_target, w, sigma, L_spatial, L_temporal):
    """Builds per-core input maps and the host-side ll constants."""
    f = np.float32
    mu = np.asarray(mu, dtype=f)
    target = np.asarray(target, dtype=f)
    unscaled_target = np.asarray(unscaled_target, dtype=f)
    Ls = np.asarray(L_spatial, dtype=np.float64)
    Lt = np.asarray(L_temporal, dtype=np.float64)

    Ks = Ls @ np.transpose(Ls, (0, 2, 1))
    Kt = Lt @ np.transpose(Lt, (0, 2, 1))
    Ds, Us = np.linalg.eigh(Ks)                   # (C, N), (C, N, N)
    Dt, Ut = np.linalg.eigh(Kt)                   # (C, T), (C, T, T)
    sig2 = np.asarray(sigma, dtype=np.float64) ** 2
    icap = 1.0 / (Dt[:, :, None] * Ds[:, None, :] + sig2[:, None, None])

    resid = (target - mu).transpose(1, 0, 2).reshape(N, B * T)      # n, (b,t)
    masku = (unscaled_target != 0).astype(np.uint8)
    masku = masku.transpose(1, 0, 2).reshape(N, B * T)

    em = np.kron(np.eye(BL, dtype=f), np.ones((T, 1), dtype=f))     # (96, 8)

    Us32 = Us.astype(f)
    icr = np.tile(icap, (1, BL, 1)).astype(f)                        # (C, 96, N)
    wblk = np.stack([np.kron(np.eye(BL), Ut[c]) for c in range(C)]).astype(f)

    Ulog = np.sum(np.log(np.einsum("cii->ci", Ls)), axis=1)
    Vlog = np.sum(np.log(np.einsum("cii->ci", Lt)), axis=1)
    logw = np.log(np.asarray(w, dtype=np.float64)[..., 0])
    m2_full = (
        -0.5 * NT * LOG2PI + N * Vlog[None, :] + T * Ulog[None, :] + logw
    ).astype(f)                                                      # (B, C)

    in_maps = []
    for k in range(NCORES):
        g, h = k // G_B, k % G_B
        comps = COMP_GROUPS[g]
        bsl = slice(h * BH * T, (h + 1) * BH * T)                    # (b,t) cols

        big = np.zeros((N, W0), dtype=f)
        for cl, c in enumerate(comps):
            big[:, cl * N : (cl + 1) * N] = Us32[c]
        big[:, RS_OFF : RS_OFF + BTL] = resid[:, bsl]
        mk = np.zeros((N, BTL), dtype=np.uint8)
        mk[:] = masku[:, bsl]
        big[:, MK_OFF:AUX_OFF] = np.ascontiguousarray(mk).view(f)
        big[0:BT, AUX_OFF : AUX_OFF + BL] = em
        big[:, AUX_OFF + BL] = 1.0

        iw = np.zeros((BT, WIK), dtype=f)
        for cl, c in enumerate(comps):
            iw[:, cl * N : (cl + 1) * N] = icr[c]
            iw[:, IC_W + cl * BT : IC_W + (cl + 1) * BT] = wblk[c]

        in_maps.append(
            {
                "in0": np.ascontiguousarray(big[0:P0]),
                "in1": np.ascontiguousarray(big[P0:N]),
                "icwk": iw,
            }
        )
    return in_maps, m2_full


def _host_final(results, m2_full):
    quad = np.zeros((B, C), dtype=np.float32)
    for k in range(NCORES):
        g, h = k // G_B, k % G_B
        comps = COMP_GROUPS[g]
        oq = results[k]["oq"]
        for cl, c in enumerate(comps):
            for q in range(NQ):
                b0 = h * BH + q * BL
                quad[b0 : b0 + BL, c] = oq[:, q * CL + cl]
    sum_abs = float(results[0]["oq"][0, NQ * CL]) + float(
        results[1]["oq"][0, NQ * CL]
    )
    sum_msk = float(results[0]["oq"][0, NQ * CL + 1]) + float(
        results[1]["oq"][0, NQ * CL + 1]
    )

    ll = m2_full - np.float32(0.5) * quad
    mx = ll.max(axis=1, keepdims=True)
    lse = np.log(np.exp(ll - mx).sum(axis=1, keepdims=True, dtype=np.float32)) + mx
    nll_loss = -np.float32(lse.sum()) / np.float32(B)
    mse_loss = np.float32(sum_abs) / np.float32(sum_msk)
    out = np.float32(RHO) * nll_loss + np.float32(1.0 - RHO) * mse_loss
    return np.asarray(out, dtype=np.float32)


def kernel(**inputs) -> np.ndarray:
    from concourse.bass_utils import run_bass_kernel_spmd

    nc = _get_program()
    in_maps, m2_full = _host_prep(
        inputs["mu"],
        inputs["target"],
        inputs["unscaled_target"],
        inputs["w"],
        inputs["sigma"],
        inputs["L_spatial"],
        inputs["L_temporal"],
    )
    res = run_bass_kernel_spmd(nc, in_maps, list(range(NCORES))).results
    return _host_final(res, m2_full)


# revision 7
# speedup vs baseline: 2.6960x; 1.9754x over previous
"""Trainium2 Bass kernel for nn_CholeskyResHead_68255620268805.

Reference math (per mixture component c of C=10):
    Ks = Ls @ Ls.T ; Kt = Lt @ Lt.T            (spatial 207x207, temporal 12x12)
    M  = (Ks  (x)  Kt + sig^2 I)^-1            (via eigh + explicit kron in ref)
    quad[b,c] = r_b^T M r_b                    (r = (target-mu).reshape(b, n*t))
    ll = -0.5*n*t*log(2pi) - 0.5*quad + n*Vlog + t*Ulog + log w
    nll_loss = mean_b(-logsumexp_c ll)
    mse_loss = mean(|mu-target| * mask/mean(mask)),  mask = (unscaled != 0)
    out = 0.1*nll_loss + 0.9*mse_loss

Key identity: with Ks = Us Ds Us^T, Kt = Ut Dt Ut^T,
    quad[b,c] = sum_{m,j} (Us^T R_b Ut)[m,j]^2 / (Ds[m] Dt[j] + sig^2)
so the (nt x nt) kron inverse never needs to be materialized.  The
temporal transform is batched through a block-diagonal kron(I_8, Ut)
stationary matrix so one matmul handles 8 batches at once.

Distribution + layout (all HW-measured on this axon/TRN2 environment):
  - per-core DMA bandwidth walls at ~78 GB/s, so per-core input BYTES are
    the whole game -> 4 component-groups x 2 batch-halves grid (3 padded
    component slots x 32 batches per core), residuals shipped host-packed
    (as the sharding hint suggests), MAE mask as uint8, and all large
    operands in fp16 (end-to-end rel err vs the fp32 reference: 7.7e-5,
    dominated by the quadratic-form terms; verified offline).
  - DMAs are split per consumer (residuals first, then per-component-slot
    eigenbasis/capacitance chunks) so compute overlaps the DMA stream.
  - components are processed in (96, 414) batch-chunk pairs to halve
    PE/ACT instruction counts.
The host does the small eigendecompositions (parameter prep, invariant
for the quadratic form), the tiny (64,10) log-sum-exp, and the final
scalar combine; the device does all batch-sized GEMM + reduction work.
"""

import numpy as np

B, N, T, C = 64, 207, 12, 10
NT = N * T
RHO = 0.1
LOG2PI = float(np.log(2.0 * np.pi))
NCORES = 8

G_B = 2                 # batch halves
G_C = 4                 # component groups
BH = B // G_B           # 32 batches per core
BTL = BH * T            # 384 (b,t) pairs per core
NQ = 4                  # batch chunks of 8 per core
NP = 2                  # chunk pairs
BL = 8                  # batches per chunk
BT = BL * T             # 96 rows per chunk
CL = 3                  # component slots per core (padded)
P0 = 128
P1 = N - P0             # 79
COMP_GROUPS = [[0, 1, 2], [3, 4, 5], [6, 7], [8, 9]]

# d16a (fp16, N rows): [ rs (BTL) | us slot 0..CL-1 (N each) ]
D16A_W = BTL + CL * N           # 1005
# d16b (fp16, BT rows): per slot [ ic (N) | wk (BT) ]
SLOT_W = N + BT                 # 303
D16B_W = CL * SLOT_W            # 909
# mk (u8, N rows): mask
# aux (f32, P0 rows): [ em (BL) | ones (1) ]

_CACHE: dict = {}


def _declare_io(nc, f32):
    import concourse.mybir as mybir

    f16 = mybir.dt.float16
    u8 = mybir.dt.uint8
    t = {}
    t["d16a"] = nc.dram_tensor("d16a", [N, D16A_W], f16, kind="ExternalInput")
    t["d16b"] = nc.dram_tensor("d16b", [BT, D16B_W], f16, kind="ExternalInput")
    t["mk"] = nc.dram_tensor("mk", [N, BTL], u8, kind="ExternalInput")
    t["aux"] = nc.dram_tensor("aux", [P0, BL + 1], f32, kind="ExternalInput")
    t["oq"] = nc.dram_tensor("oq", [BL, NQ * CL + 2], f32, kind="ExternalOutput")
    return t


def _emit_body(nc, tc, io):
    import concourse.mybir as mybir

    f32 = mybir.dt.float32
    f16 = mybir.dt.float16
    u8 = mybir.dt.uint8
    AF = mybir.ActivationFunctionType
    OP = mybir.AluOpType
    AX = mybir.AxisListType

    with (
        tc.tile_pool(name="cst", bufs=1) as cst,
        tc.tile_pool(name="ztp", bufs=3) as ztp,
        tc.tile_pool(name="sqp", bufs=3) as sqp,
        tc.tile_pool(name="scp", bufs=2) as scp,
        tc.tile_pool(name="ps_z", bufs=3, space="PSUM") as ps_z,
        tc.tile_pool(name="ps_y", bufs=2, space="PSUM") as ps_y,
        tc.tile_pool(name="ps_s", bufs=1, space="PSUM") as ps_s,
    ):
        # ---- loads, consumer-ordered, split across both HWDGE rings ----
        rs0 = cst.tile([P0, BTL], f16, tag="rs0")
        rs1 = cst.tile([P1, BTL], f16, tag="rs1")
        nc.sync.dma_start(rs0[:], io["d16a"][0:P0, 0:BTL])
        nc.scalar.dma_start(rs1[:], io["d16a"][P0:N, 0:BTL])
        mk0 = cst.tile([P0, BTL], u8, tag="mk0")
        mk1 = cst.tile([P1, BTL], u8, tag="mk1")
        nc.sync.dma_start(mk0[:], io["mk"][0:P0, :])
        nc.scalar.dma_start(mk1[:], io["mk"][P0:N, :])
        aux = cst.tile([P0, BL + 1], f32, tag="aux")
        nc.sync.dma_start(aux[:], io["aux"][:])

        us0 = []
        us1 = []
        icw = []
        for cl in range(CL):
            a = cst.tile([P0, N], f16, tag=f"us0_{cl}", name=f"us0_{cl}")
            b = cst.tile([P1, N], f16, tag=f"us1_{cl}", name=f"us1_{cl}")
            w = cst.tile([BT, SLOT_W], f16, tag=f"icw_{cl}", name=f"icw_{cl}")
            off = BTL + cl * N
            nc.sync.dma_start(a[:], io["d16a"][0:P0, off : off + N])
            nc.scalar.dma_start(b[:], io["d16a"][P0:N, off : off + N])
            nc.scalar.dma_start(w[:], io["d16b"][:, cl * SLOT_W : (cl + 1) * SLOT_W])
            us0.append(a)
            us1.append(b)
            icw.append(w)

        emt = aux[0:BT, 0:BL]
        onest = aux[:, BL : BL + 1]

        # ---- masked-MAE partial sums ----
        mkf0 = cst.tile([P0, BTL], f16, tag="mkf0")
        mkf1 = cst.tile([P1, BTL], f16, tag="mkf1")
        nc.vector.tensor_copy(mkf0[:], mk0[:])
        nc.vector.tensor_copy(mkf1[:], mk1[:])
        mr0 = cst.tile([P0, BTL], f16, tag="mr0")
        mr1 = cst.tile([P1, BTL], f16, tag="mr1")
        nc.vector.tensor_mul(mr0[:], rs0[:], mkf0[:])
        nc.vector.tensor_mul(mr1[:], rs1[:], mkf1[:])
        pt0 = cst.tile([P0, 2], f32, tag="pt0")
        pt1 = cst.tile([P1, 2], f32, tag="pt1")
        nc.vector.tensor_reduce(
            pt0[:, 0:1], mr0[:], axis=AX.X, op=OP.add, apply_absolute_value=True
        )
        nc.vector.tensor_reduce(pt0[:, 1:2], mkf0[:], axis=AX.X, op=OP.add)
        nc.vector.tensor_reduce(
            pt1[:, 0:1], mr1[:], axis=AX.X, op=OP.add, apply_absolute_value=True
        )
        nc.vector.tensor_reduce(pt1[:, 1:2], mkf1[:], axis=AX.X, op=OP.add)
        mae_ps = ps_s.tile([1, 2], f32, tag="mae_ps")
        nc.tensor.matmul(mae_ps[:], onest, pt0[:], start=True, stop=False)
        nc.tensor.matmul(mae_ps[:], onest[0:P1, :], pt1[:], start=False, stop=True)

        # ---- per-(slot, chunk-pair) quadratic forms ----
        # S[(b,j), q*CL+cl] = sum_m (Us^T R_b Ut)[m,j]^2 * icap[j,m]
        S = cst.tile([BT, NQ * CL], f32, tag="S")
        for cl in range(CL):
            ict = icw[cl][:, 0:N]
            wkt = icw[cl][:, N : N + BT]
            for p in range(NP):
                q0, q1 = 2 * p, 2 * p + 1
                zt = ps_z.tile([BT, 2 * N], f32, tag="zt")
                for hi, q in ((0, q0), (1, q1)):
                    cz = zt[:, hi * N : (hi + 1) * N]
                    nc.tensor.matmul(
                        cz, rs0[:, q * BT : (q + 1) * BT], us0[cl][:],
                        start=True, stop=False,
                    )
                    nc.tensor.matmul(
                        cz, rs1[:, q * BT : (q + 1) * BT], us1[cl][:],
                        start=False, stop=True,
                    )
                ztsb = ztp.tile([BT, 2 * N], f16, tag="ztsb")
                if (cl * NP + p) % 2 == 0:
                    nc.scalar.copy(ztsb[:], zt[:])
                else:
                    nc.vector.tensor_copy(ztsb[:], zt[:])

                yt = ps_y.tile([BT, 2 * N], f32, tag="yt")
                nc.tensor.matmul(yt[:], wkt, ztsb[:], start=True, stop=True)
                sq = sqp.tile([BT, 2 * N], f32, tag="sq")
                nc.scalar.activation(sq[:], yt[:], AF.Square)
                for hi, q in ((0, q0), (1, q1)):
                    scr = scp.tile([BT, N], f32, tag="scr")
                    # scr = (sq * 1.0) * ic ; S[:,col] = sum_m scr
                    nc.vector.scalar_tensor_tensor(
                        scr[:],
                        sq[:, hi * N : (hi + 1) * N],
                        1.0,
                        ict,
                        op0=OP.mult,
                        op1=OP.mult,
                        accum_out=S[:, q * CL + cl : q * CL + cl + 1],
                    )

        # ---- quad[b, (q,cl)] = sum_j S[(b,j), (q,cl)] ----
        q_ps = ps_s.tile([BL, NQ * CL], f32, tag="q_ps")
        nc.tensor.matmul(q_ps[:], emt, S[:], start=True, stop=True)

        # ---- pack outputs: quad (8, 12) + [mae_abs, mae_cnt] on row 0 ----
        ot = cst.tile([BL, NQ * CL + 2], f32, tag="ot")
        nc.scalar.copy(ot[:, 0 : NQ * CL], q_ps[:])
        nc.vector.tensor_scalar(
            ot[:, NQ * CL : NQ * CL + 2], q_ps[:, 0:2], 0.0, None, op0=OP.mult
        )
        nc.scalar.copy(ot[0:1, NQ * CL : NQ * CL + 2], mae_ps[:])
        nc.sync.dma_start(io["oq"][:], ot[:])


def _build_program():
    import concourse.bacc as bacc
    import concourse.mybir as mybir
    from concourse import tile

    f32 = mybir.dt.float32
    nc = bacc.Bacc(None, target_bir_lowering=False)
    io = _declare_io(nc, f32)
    with tile.TileContext(nc) as tc:
        _emit_body(nc, tc, io)
    nc.compile()
    return nc


def _get_program():
    if "nc" not in _CACHE:
        _CACHE["nc"] = _build_program()
    return _CACHE["nc"]


def _host_prep(mu, target, unscaled_target, w, sigma, L_spatial, L_temporal):
    """Builds per-core input maps and the host-side ll constants."""
    f = np.float32
    h = np.float16
    mu = np.asarray(mu, dtype=f)
    target = np.asarray(target, dtype=f)
    unscaled_target = np.asarray(unscaled_target, dtype=f)
    Ls = np.asarray(L_spatial, dtype=np.float64)
    Lt = np.asarray(L_temporal, dtype=np.float64)

    Ks = Ls @ np.transpose(Ls, (0, 2, 1))
    Kt = Lt @ np.transpose(Lt, (0, 2, 1))
    Ds, Us = np.linalg.eigh(Ks)                   # (C, N), (C, N, N)
    Dt, Ut = np.linalg.eigh(Kt)                   # (C, T), (C, T, T)
    sig2 = np.asarray(sigma, dtype=np.float64) ** 2
    icap = 1.0 / (Dt[:, :, None] * Ds[:, None, :] + sig2[:, None, None])

    resid = (target - mu).transpose(1, 0, 2).reshape(N, B * T)      # n, (b,t)
    masku = (unscaled_target != 0).astype(np.uint8)
    masku = masku.transpose(1, 0, 2).reshape(N, B * T)

    em = np.kron(np.eye(BL, dtype=f), np.ones((T, 1), dtype=f))     # (96, 8)
    Us16 = Us.astype(h)
    ic16 = np.tile(icap, (1, BL, 1)).astype(h)                       # (C, 96, N)
    wk16 = np.stack([np.kron(np.eye(BL), Ut[c]) for c in range(C)]).astype(h)

    Ulog = np.sum(np.log(np.einsum("cii->ci", Ls)), axis=1)
    Vlog = np.sum(np.log(np.einsum("cii->ci", Lt)), axis=1)
    logw = np.log(np.asarray(w, dtype=np.float64)[..., 0])
    m2_full = (
        -0.5 * NT * LOG2PI + N * Vlog[None, :] + T * Ulog[None, :] + logw
    ).astype(f)                                                      # (B, C)

    aux = np.zeros((P0, BL + 1), dtype=f)
    aux[0:BT, 0:BL] = em
    aux[:, BL] = 1.0

    in_maps = []
    for k in range(NCORES):
        g, hh = k // G_B, k % G_B
        comps = COMP_GROUPS[g]
        bsl = slice(hh * BTL, (hh + 1) * BTL)

        d16a = np.zeros((N, D16A_W), dtype=h)
        d16a[:, 0:BTL] = resid[:, bsl].astype(h)
        for cl, c in enumerate(comps):
            d16a[:, BTL + cl * N : BTL + (cl + 1) * N] = Us16[c]
        d16b = np.zeros((BT, D16B_W), dtype=h)
        for cl, c in enumerate(comps):
            d16b[:, cl * SLOT_W : cl * SLOT_W + N] = ic16[c]
            d16b[:, cl * SLOT_W + N : (cl + 1) * SLOT_W] = wk16[c]

        in_maps.append(
            {
                "d16a": d16a,
                "d16b": d16b,
                "mk": np.ascontiguousarray(masku[:, bsl]),
                "aux": aux,
            }
        )
    return in_maps, m2_full


def _host_final(results, m2_full):
    quad = np.zeros((B, C), dtype=np.float32)
    for k in range(NCORES):
        g, h = k // G_B, k % G_B
        comps = COMP_GROUPS[g]
        oq = results[k]["oq"]
        for cl, c in enumerate(comps):
            for q in range(NQ):
                b0 = h * BH + q * BL
                quad[b0 : b0 + BL, c] = oq[:, q * CL + cl]
    sum_abs = float(results[0]["oq"][0, NQ * CL]) + float(
        results[1]["oq"][0, NQ * CL]
    )
    sum_msk = float(results[0]["oq"][0, NQ * CL + 1]) + float(
        results[1]["oq"][0, NQ * CL + 1]
    )

    ll = m2_full - np.float32(0.5) * quad
    mx = ll.max(axis=1, keepdims=True)
    lse = np.log(np.exp(ll - mx).sum(axis=1, keepdims=True, dtype=np.float32)) + mx
    nll_loss = -np.float32(lse.sum()) / np.float32(B)
    mse_loss = np.float32(sum_abs) / np.float32(sum_msk)
    out = np.float32(RHO) * nll_loss + np.float32(1.0 - RHO) * mse_loss
    return np.asarray(out, dtype=np.float32)


def kernel(**inputs) -> np.ndarray:
    from concourse.bass_utils import run_bass_kernel_spmd

    nc = _get_program()
    in_maps, m2_full = _host_prep(
        inputs["mu"],
        inputs["target"],
        inputs["unscaled_target"],
        inputs["w"],
        inputs["sigma"],
        inputs["L_spatial"],
        inputs["L_temporal"],
    )
    res = run_bass_kernel_spmd(nc, in_maps, list(range(NCORES))).results
    return _host_final(res, m2_full)


# revision 8
# speedup vs baseline: 2.7051x; 1.0034x over previous
"""Trainium2 Bass kernel for nn_CholeskyResHead_68255620268805.

Reference math (per mixture component c of C=10):
    Ks = Ls @ Ls.T ; Kt = Lt @ Lt.T            (spatial 207x207, temporal 12x12)
    M  = (Ks  (x)  Kt + sig^2 I)^-1            (via eigh + explicit kron in ref)
    quad[b,c] = r_b^T M r_b                    (r = (target-mu).reshape(b, n*t))
    ll = -0.5*n*t*log(2pi) - 0.5*quad + n*Vlog + t*Ulog + log w
    nll_loss = mean_b(-logsumexp_c ll)
    mse_loss = mean(|mu-target| * mask/mean(mask)),  mask = (unscaled != 0)
    out = 0.1*nll_loss + 0.9*mse_loss

Key identity: with Ks = Us Ds Us^T, Kt = Ut Dt Ut^T,
    quad[b,c] = sum_{m,j} (Us^T R_b Ut)[m,j]^2 / (Ds[m] Dt[j] + sig^2)
so the (nt x nt) kron inverse never needs to be materialized.  The
temporal transform is batched through a block-diagonal kron(I_8, Ut)
stationary matrix so one matmul handles 8 batches at once.

Distribution + layout (all HW-measured on this axon/TRN2 environment):
  - per-core DMA bandwidth walls at ~78 GB/s, so per-core input BYTES are
    the whole game -> 4 component-groups x 2 batch-halves grid (3 padded
    component slots x 32 batches per core), residuals shipped host-packed
    (as the sharding hint suggests), MAE mask as uint8, and all large
    operands in fp16 (end-to-end rel err vs the fp32 reference: 7.7e-5,
    dominated by the quadratic-form terms; verified offline).
  - DMAs are split per consumer (residuals first, then per-component-slot
    eigenbasis/capacitance chunks) so compute overlaps the DMA stream.
  - components are processed in (96, 414) batch-chunk pairs to halve
    PE/ACT instruction counts.
The host does the small eigendecompositions (parameter prep, invariant
for the quadratic form), the tiny (64,10) log-sum-exp, and the final
scalar combine; the device does all batch-sized GEMM + reduction work.
"""

import numpy as np

B, N, T, C = 64, 207, 12, 10
NT = N * T
RHO = 0.1
LOG2PI = float(np.log(2.0 * np.pi))
NCORES = 8

G_B = 2                 # batch halves
G_C = 4                 # component groups
BH = B // G_B           # 32 batches per core
BTL = BH * T            # 384 (b,t) pairs per core
NQ = 4                  # batch chunks of 8 per core
NP = 2                  # chunk pairs
BL = 8                  # batches per chunk
BT = BL * T             # 96 rows per chunk
CL = 3                  # component slots per core (padded)
P0 = 128
P1 = N - P0             # 79
COMP_GROUPS = [[0, 1, 2], [3, 4, 5], [6, 7], [8, 9]]

# d16a (fp16, N rows): [ rs (BTL) | us slot 0..CL-1 (N each) ]
D16A_W = BTL + CL * N           # 1005
# d16b (fp16, BT rows): per slot [ ic (N) | wk (BT) ]
SLOT_W = N + BT                 # 303
D16B_W = CL * SLOT_W            # 909
# mk (u8, N rows): mask
# aux (f32, P0 rows): [ em (BL) | ones (1) ]

_CACHE: dict = {}
ABLATE = None  # None | "loads" | "nostage2" | "nomae" | "stage1only"


def _declare_io(nc, f32):
    import concourse.mybir as mybir

    f16 = mybir.dt.float16
    u8 = mybir.dt.uint8
    t = {}
    t["d16a"] = nc.dram_tensor("d16a", [N, D16A_W], f16, kind="ExternalInput")
    t["d16b"] = nc.dram_tensor("d16b", [BT, D16B_W], f16, kind="ExternalInput")
    t["mk"] = nc.dram_tensor("mk", [N, BTL], u8, kind="ExternalInput")
    t["aux"] = nc.dram_tensor("aux", [P0, BL + 1], f32, kind="ExternalInput")
    t["oq"] = nc.dram_tensor("oq", [BL, NQ * CL + 2], f32, kind="ExternalOutput")
    return t


def _emit_body(nc, tc, io):
    import concourse.mybir as mybir

    f32 = mybir.dt.float32
    f16 = mybir.dt.float16
    u8 = mybir.dt.uint8
    AF = mybir.ActivationFunctionType
    OP = mybir.AluOpType
    AX = mybir.AxisListType

    with (
        tc.tile_pool(name="cst", bufs=1) as cst,
        tc.tile_pool(name="ztp", bufs=3) as ztp,
        tc.tile_pool(name="sqp", bufs=3) as sqp,
        tc.tile_pool(name="scp", bufs=2) as scp,
        tc.tile_pool(name="ps_z", bufs=3, space="PSUM") as ps_z,
        tc.tile_pool(name="ps_y", bufs=2, space="PSUM") as ps_y,
        tc.tile_pool(name="ps_s", bufs=1, space="PSUM") as ps_s,
    ):
        # ---- loads, consumer-ordered, split across both HWDGE rings ----
        rs0 = cst.tile([P0, BTL], f16, tag="rs0")
        rs1 = cst.tile([P1, BTL], f16, tag="rs1")
        nc.sync.dma_start(rs0[:], io["d16a"][0:P0, 0:BTL])
        nc.scalar.dma_start(rs1[:], io["d16a"][P0:N, 0:BTL])
        mk0 = cst.tile([P0, BTL], u8, tag="mk0")
        mk1 = cst.tile([P1, BTL], u8, tag="mk1")
        nc.sync.dma_start(mk0[:], io["mk"][0:P0, :])
        nc.scalar.dma_start(mk1[:], io["mk"][P0:N, :])
        aux = cst.tile([P0, BL + 1], f32, tag="aux")
        nc.sync.dma_start(aux[:], io["aux"][:])

        us0 = []
        us1 = []
        icw = []
        for cl in range(CL):
            a = cst.tile([P0, N], f16, tag=f"us0_{cl}", name=f"us0_{cl}")
            b = cst.tile([P1, N], f16, tag=f"us1_{cl}", name=f"us1_{cl}")
            w = cst.tile([BT, SLOT_W], f16, tag=f"icw_{cl}", name=f"icw_{cl}")
            off = BTL + cl * N
            nc.sync.dma_start(a[:], io["d16a"][0:P0, off : off + N])
            nc.scalar.dma_start(b[:], io["d16a"][P0:N, off : off + N])
            nc.scalar.dma_start(w[:], io["d16b"][:, cl * SLOT_W : (cl + 1) * SLOT_W])
            us0.append(a)
            us1.append(b)
            icw.append(w)

        emt = aux[0:BT, 0:BL]
        onest = aux[:, BL : BL + 1]

        if ABLATE == "loads":
            ot = cst.tile([BL, NQ * CL + 2], f32, tag="ot")
            nc.vector.tensor_scalar(
                ot[:], icw[0][0:BL, 0 : NQ * CL + 2], 0.0, None, op0=OP.mult
            )
            nc.sync.dma_start(io["oq"][:], ot[:])
            return

        # ---- masked-MAE partial sums ----
        skip_mae = ABLATE == "nomae"
        mkf0 = cst.tile([P0, BTL], f16, tag="mkf0")
        mkf1 = cst.tile([P1, BTL], f16, tag="mkf1")
        if not skip_mae:
            nc.vector.tensor_copy(mkf0[:], mk0[:])
            nc.vector.tensor_copy(mkf1[:], mk1[:])
            mr0 = cst.tile([P0, BTL], f16, tag="mr0")
            mr1 = cst.tile([P1, BTL], f16, tag="mr1")
            nc.vector.tensor_mul(mr0[:], rs0[:], mkf0[:])
            nc.vector.tensor_mul(mr1[:], rs1[:], mkf1[:])
            pt0 = cst.tile([P0, 2], f32, tag="pt0")
            pt1 = cst.tile([P1, 2], f32, tag="pt1")
            nc.vector.tensor_reduce(
                pt0[:, 0:1], mr0[:], axis=AX.X, op=OP.add, apply_absolute_value=True
            )
            nc.vector.tensor_reduce(pt0[:, 1:2], mkf0[:], axis=AX.X, op=OP.add)
            nc.vector.tensor_reduce(
                pt1[:, 0:1], mr1[:], axis=AX.X, op=OP.add, apply_absolute_value=True
            )
            nc.vector.tensor_reduce(pt1[:, 1:2], mkf1[:], axis=AX.X, op=OP.add)
        mae_ps = ps_s.tile([1, 2], f32, tag="mae_ps")
        if not skip_mae:
            nc.tensor.matmul(mae_ps[:], onest, pt0[:], start=True, stop=False)
            nc.tensor.matmul(mae_ps[:], onest[0:P1, :], pt1[:], start=False, stop=True)
        else:
            nc.tensor.matmul(mae_ps[:], onest[0:2, :], aux[0:2, 0:2], start=True, stop=True)

        # ---- per-(slot, chunk-pair) quadratic forms ----
        # S[(b,j), q*CL+cl] = sum_m (Us^T R_b Ut)[m,j]^2 * icap[j,m]
        S = cst.tile([BT, NQ * CL], f32, tag="S")
        for cl in range(CL):
            ict = icw[cl][:, 0:N]
            wkt = icw[cl][:, N : N + BT]
            for p in range(NP):
                q0, q1 = 2 * p, 2 * p + 1
                zt = ps_z.tile([BT, 2 * N], f32, tag="zt")
                for hi, q in ((0, q0), (1, q1)):
                    cz = zt[:, hi * N : (hi + 1) * N]
                    nc.tensor.matmul(
                        cz, rs0[:, q * BT : (q + 1) * BT], us0[cl][:],
                        start=True, stop=False,
                    )
                    nc.tensor.matmul(
                        cz, rs1[:, q * BT : (q + 1) * BT], us1[cl][:],
                        start=False, stop=True,
                    )
                ztsb = ztp.tile([BT, 2 * N], f16, tag="ztsb")
                if (cl * NP + p) % 2 == 0:
                    nc.scalar.copy(ztsb[:], zt[:])
                else:
                    nc.vector.tensor_copy(ztsb[:], zt[:])
                if ABLATE in ("stage1only", "nostage2"):
                    nc.vector.scalar_tensor_tensor(
                        mkf0[0:BT, 0:N], ztsb[:, 0:N], 1.0, ict,
                        op0=OP.mult, op1=OP.mult,
                        accum_out=S[:, (2 * p) * CL + cl : (2 * p) * CL + cl + 1],
                    ) if ABLATE == "nostage2" else None
                    if ABLATE == "stage1only":
                        nc.vector.tensor_scalar(
                            S[:, (2 * p) * CL + cl : (2 * p) * CL + cl + 1],
                            ztsb[:, 0:1], 1.0, None, op0=OP.mult)
                    else:
                        nc.vector.tensor_scalar(
                            S[:, (2 * p + 1) * CL + cl : (2 * p + 1) * CL + cl + 1],
                            ztsb[:, 0:1], 1.0, None, op0=OP.mult)
                    continue

                yt = ps_y.tile([BT, 2 * N], f32, tag="yt")
                nc.tensor.matmul(yt[:], wkt, ztsb[:], start=True, stop=True)
                sq = sqp.tile([BT, 2 * N], f32, tag="sq")
                nc.scalar.activation(sq[:], yt[:], AF.Square)
                for hi, q in ((0, q0), (1, q1)):
                    scr = scp.tile([BT, N], f32, tag="scr")
                    # scr = (sq * 1.0) * ic ; S[:,col] = sum_m scr
                    nc.vector.scalar_tensor_tensor(
                        scr[:],
                        sq[:, hi * N : (hi + 1) * N],
                        1.0,
                        ict,
                        op0=OP.mult,
                        op1=OP.mult,
                        accum_out=S[:, q * CL + cl : q * CL + cl + 1],
                    )

        # ---- quad[b, (q,cl)] = sum_j S[(b,j), (q,cl)] ----
        q_ps = ps_s.tile([BL, NQ * CL], f32, tag="q_ps")
        nc.tensor.matmul(q_ps[:], emt, S[:], start=True, stop=True)

        # ---- pack outputs: quad (8, 12) + [mae_abs, mae_cnt] on row 0 ----
        ot = cst.tile([BL, NQ * CL + 2], f32, tag="ot")
        nc.scalar.copy(ot[:, 0 : NQ * CL], q_ps[:])
        nc.vector.tensor_scalar(
            ot[:, NQ * CL : NQ * CL + 2], q_ps[:, 0:2], 0.0, None, op0=OP.mult
        )
        nc.scalar.copy(ot[0:1, NQ * CL : NQ * CL + 2], mae_ps[:])
        nc.sync.dma_start(io["oq"][:], ot[:])


def _build_program():
    import concourse.bacc as bacc
    import concourse.mybir as mybir
    from concourse import tile

    f32 = mybir.dt.float32
    nc = bacc.Bacc(None, target_bir_lowering=False)
    io = _declare_io(nc, f32)
    with tile.TileContext(nc) as tc:
        _emit_body(nc, tc, io)
    nc.compile()
    return nc


def _get_program():
    if "nc" not in _CACHE:
        _CACHE["nc"] = _build_program()
    return _CACHE["nc"]


def _host_prep(mu, target, unscaled_target, w, sigma, L_spatial, L_temporal):
    """Builds per-core input maps and the host-side ll constants."""
    f = np.float32
    h = np.float16
    mu = np.asarray(mu, dtype=f)
    target = np.asarray(target, dtype=f)
    unscaled_target = np.asarray(unscaled_target, dtype=f)
    Ls = np.asarray(L_spatial, dtype=np.float64)
    Lt = np.asarray(L_temporal, dtype=np.float64)

    Ks = Ls @ np.transpose(Ls, (0, 2, 1))
    Kt = Lt @ np.transpose(Lt, (0, 2, 1))
    Ds, Us = np.linalg.eigh(Ks)                   # (C, N), (C, N, N)
    Dt, Ut = np.linalg.eigh(Kt)                   # (C, T), (C, T, T)
    sig2 = np.asarray(sigma, dtype=np.float64) ** 2
    icap = 1.0 / (Dt[:, :, None] * Ds[:, None, :] + sig2[:, None, None])

    resid = (target - mu).transpose(1, 0, 2).reshape(N, B * T)      # n, (b,t)
    masku = (unscaled_target != 0).astype(np.uint8)
    masku = masku.transpose(1, 0, 2).reshape(N, B * T)

    em = np.kron(np.eye(BL, dtype=f), np.ones((T, 1), dtype=f))     # (96, 8)
    Us16 = Us.astype(h)
    ic16 = np.tile(icap, (1, BL, 1)).astype(h)                       # (C, 96, N)
    wk16 = np.stack([np.kron(np.eye(BL), Ut[c]) for c in range(C)]).astype(h)

    Ulog = np.sum(np.log(np.einsum("cii->ci", Ls)), axis=1)
    Vlog = np.sum(np.log(np.einsum("cii->ci", Lt)), axis=1)
    logw = np.log(np.asarray(w, dtype=np.float64)[..., 0])
    m2_full = (
        -0.5 * NT * LOG2PI + N * Vlog[None, :] + T * Ulog[None, :] + logw
    ).astype(f)                                                      # (B, C)

    aux = np.zeros((P0, BL + 1), dtype=f)
    aux[0:BT, 0:BL] = em
    aux[:, BL] = 1.0

    in_maps = []
    for k in range(NCORES):
        g, hh = k // G_B, k % G_B
        comps = COMP_GROUPS[g]
        bsl = slice(hh * BTL, (hh + 1) * BTL)

        d16a = np.zeros((N, D16A_W), dtype=h)
        d16a[:, 0:BTL] = resid[:, bsl].astype(h)
        for cl, c in enumerate(comps):
            d16a[:, BTL + cl * N : BTL + (cl + 1) * N] = Us16[c]
        d16b = np.zeros((BT, D16B_W), dtype=h)
        for cl, c in enumerate(comps):
            d16b[:, cl * SLOT_W : cl * SLOT_W + N] = ic16[c]
            d16b[:, cl * SLOT_W + N : (cl + 1) * SLOT_W] = wk16[c]

        in_maps.append(
            {
                "d16a": d16a,
                "d16b": d16b,
                "mk": np.ascontiguousarray(masku[:, bsl]),
                "aux": aux,
            }
        )
    return in_maps, m2_full


def _host_final(results, m2_full):
    quad = np.zeros((B, C), dtype=np.float32)
    for k in range(NCORES):
        g, h = k // G_B, k % G_B
        comps = COMP_GROUPS[g]
        oq = results[k]["oq"]
        for cl, c in enumerate(comps):
            for q in range(NQ):
                b0 = h * BH + q * BL
                quad[b0 : b0 + BL, c] = oq[:, q * CL + cl]
    sum_abs = float(results[0]["oq"][0, NQ * CL]) + float(
        results[1]["oq"][0, NQ * CL]
    )
    sum_msk = float(results[0]["oq"][0, NQ * CL + 1]) + float(
        results[1]["oq"][0, NQ * CL + 1]
    )

    ll = m2_full - np.float32(0.5) * quad
    mx = ll.max(axis=1, keepdims=True)
    lse = np.log(np.exp(ll - mx).sum(axis=1, keepdims=True, dtype=np.float32)) + mx
    nll_loss = -np.float32(lse.sum()) / np.float32(B)
    mse_loss = np.float32(sum_abs) / np.float32(sum_msk)
    out = np.float32(RHO) * nll_loss + np.float32(1.0 - RHO) * mse_loss
    return np.asarray(out, dtype=np.float32)


def kernel(**inputs) -> np.ndarray:
    from concourse.bass_utils import run_bass_kernel_spmd

    nc = _get_program()
    in_maps, m2_full = _host_prep(
        inputs["mu"],
        inputs["target"],
        inputs["unscaled_target"],
        inputs["w"],
        inputs["sigma"],
        inputs["L_spatial"],
        inputs["L_temporal"],
    )
    res = run_bass_kernel_spmd(nc, in_maps, list(range(NCORES))).results
    return _host_final(res, m2_full)


# revision 9
# speedup vs baseline: 2.7181x; 1.0048x over previous
"""Trainium2 Bass kernel for nn_CholeskyResHead_68255620268805.

Reference math (per mixture component c of C=10):
    Ks = Ls @ Ls.T ; Kt = Lt @ Lt.T            (spatial 207x207, temporal 12x12)
    M  = (Ks  (x)  Kt + sig^2 I)^-1            (via eigh + explicit kron in ref)
    quad[b,c] = r_b^T M r_b                    (r = (target-mu).reshape(b, n*t))
    ll = -0.5*n*t*log(2pi) - 0.5*quad + n*Vlog + t*Ulog + log w
    nll_loss = mean_b(-logsumexp_c ll)
    mse_loss = mean(|mu-target| * mask/mean(mask)),  mask = (unscaled != 0)
    out = 0.1*nll_loss + 0.9*mse_loss

Key identity: with Ks = Us Ds Us^T, Kt = Ut Dt Ut^T,
    quad[b,c] = sum_{m,j} (Us^T R_b Ut)[m,j]^2 / (Ds[m] Dt[j] + sig^2)
so the (nt x nt) kron inverse never needs to be materialized.  The
temporal transform is batched through a block-diagonal kron(I_8, Ut)
stationary matrix so one matmul handles 8 batches at once.

Distribution + layout (HW-measured on this axon/TRN2 environment):
  - per-core DMA bandwidth walls at ~78 GB/s with ~0.5 us marginal cost
    per dma_start, so per-core input bytes AND dma count are the game:
    4 component-groups x 2 batch-halves grid (3 padded component slots x
    32 batches per core), residuals shipped host-packed (as the sharding
    hint suggests), MAE mask bit-packed as u8 inside the fp16 stream, and
    all large operands fp16 (end-to-end rel err vs the fp32 reference:
    7.7e-5, verified offline; the quadratic form is robust to parameter
    rounding because the decomposition identity is exact for whatever
    rounded operands are used consistently).
  - 8 input DMAs issued from the sync sequencer and the otherwise-idle
    gpsimd sequencer - never from ACT/DVE, whose queues are the critical
    path (DMA issue costs ~0.5 us of issuing-engine time).
  - components are processed in (96, 414) batch-chunk pairs to halve
    PE/ACT instruction counts; PSUM->SBUF eigencoefficient copies
    alternate between ACT and DVE.
The host does the small eigendecompositions (parameter prep, invariant
for the quadratic form), the tiny (64,10) log-sum-exp, and the final
scalar combine; the device does all batch-sized GEMM + reduction work.
"""

import numpy as np

B, N, T, C = 64, 207, 12, 10
NT = N * T
RHO = 0.1
LOG2PI = float(np.log(2.0 * np.pi))
NCORES = 8

G_B = 2                 # batch halves
G_C = 4                 # component groups
BH = B // G_B           # 32 batches per core
BTL = BH * T            # 384 (b,t) pairs per core
NQ = 4                  # batch chunks of 8 per core
NP = 2                  # chunk pairs
BL = 8                  # batches per chunk
BT = BL * T             # 96 rows per chunk
CL = 3                  # component slots per core (padded)
P0 = 128
P1 = N - P0             # 79
COMP_GROUPS = [[0, 1, 2], [3, 4, 5], [6, 7], [8, 9]]

# d16a (fp16, N rows): [ rs (BTL) | mask-as-f16 (BTL/2) | us slots (N each) ]
MK_OFF = BTL            # 384
US_OFF = BTL + BTL // 2  # 576
D16A_W = US_OFF + CL * N  # 1197
# d16b (fp16, BT rows): per slot [ ic (N) | wk (BT) ]
SLOT_W = N + BT         # 303
D16B_W = CL * SLOT_W    # 909
# aux (f32, P0 rows): [ em (BL) | ones (1) ]

_CACHE: dict = {}
ABLATE = None


def _declare_io(nc, f32):
    import concourse.mybir as mybir

    f16 = mybir.dt.float16
    t = {}
    t["d16a"] = nc.dram_tensor("d16a", [N, D16A_W], f16, kind="ExternalInput")
    t["d16b"] = nc.dram_tensor("d16b", [BT, D16B_W], f16, kind="ExternalInput")
    t["aux"] = nc.dram_tensor("aux", [P0, BL + 1], f32, kind="ExternalInput")
    t["oq"] = nc.dram_tensor("oq", [BL, NQ * CL + 2], f32, kind="ExternalOutput")
    return t


def _emit_body(nc, tc, io):
    import concourse.mybir as mybir

    f32 = mybir.dt.float32
    f16 = mybir.dt.float16
    u8 = mybir.dt.uint8
    AF = mybir.ActivationFunctionType
    OP = mybir.AluOpType
    AX = mybir.AxisListType

    with (
        tc.tile_pool(name="cst", bufs=1) as cst,
        tc.tile_pool(name="ztp", bufs=3) as ztp,
        tc.tile_pool(name="sqp", bufs=3) as sqp,
        tc.tile_pool(name="scp", bufs=2) as scp,
        tc.tile_pool(name="ps_z", bufs=3, space="PSUM") as ps_z,
        tc.tile_pool(name="ps_y", bufs=2, space="PSUM") as ps_y,
        tc.tile_pool(name="ps_s", bufs=1, space="PSUM") as ps_s,
    ):
        # ---- loads: sync ring gets the 128-row half, gpsimd (SWDGE, idle
        # ---- engine) gets the 79-row half + component params ----
        rm0 = cst.tile([P0, US_OFF], f16, tag="rm0")
        rm1 = cst.tile([P1, US_OFF], f16, tag="rm1")
        nc.sync.dma_start(rm0[:], io["d16a"][0:P0, 0:US_OFF])
        nc.gpsimd.dma_start(rm1[:], io["d16a"][P0:N, 0:US_OFF])
        us0 = cst.tile([P0, CL * N], f16, tag="us0")
        us1 = cst.tile([P1, CL * N], f16, tag="us1")
        nc.sync.dma_start(us0[:], io["d16a"][0:P0, US_OFF:D16A_W])
        nc.gpsimd.dma_start(us1[:], io["d16a"][P0:N, US_OFF:D16A_W])
        aux = cst.tile([P0, BL + 1], f32, tag="aux")
        nc.sync.dma_start(aux[:], io["aux"][:])
        icw = []
        for cl in range(CL):
            w = cst.tile([BT, SLOT_W], f16, tag=f"icw_{cl}", name=f"icw_{cl}")
            nc.gpsimd.dma_start(
                w[:], io["d16b"][:, cl * SLOT_W : (cl + 1) * SLOT_W]
            )
            icw.append(w)

        rs0 = rm0[:, 0:BTL]
        rs1 = rm1[:, 0:BTL]
        mk0 = rm0[:, MK_OFF:US_OFF].bitcast(u8)
        mk1 = rm1[:, MK_OFF:US_OFF].bitcast(u8)
        emt = aux[0:BT, 0:BL]
        onest = aux[:, BL : BL + 1]

        # ---- masked-MAE partial sums ----
        # pack [mr | mask] side by side so one 3D reduce per chunk covers both
        mm0 = cst.tile([P0, 2, BTL], f16, tag="mm0")
        mm1 = cst.tile([P1, 2, BTL], f16, tag="mm1")
        nc.vector.tensor_copy(mm0[:, 1, :], mk0)
        nc.vector.tensor_copy(mm1[:, 1, :], mk1)
        nc.vector.tensor_mul(mm0[:, 0, :], rs0, mm0[:, 1, :])
        nc.vector.tensor_mul(mm1[:, 0, :], rs1, mm1[:, 1, :])
        pt0 = cst.tile([P0, 2], f32, tag="pt0")
        pt1 = cst.tile([P1, 2], f32, tag="pt1")
        nc.vector.tensor_reduce(
            pt0[:], mm0[:], axis=AX.X, op=OP.add, apply_absolute_value=True
        )
        nc.vector.tensor_reduce(
            pt1[:], mm1[:], axis=AX.X, op=OP.add, apply_absolute_value=True
        )
        mae_ps = ps_s.tile([1, 2], f32, tag="mae_ps")
        nc.tensor.matmul(mae_ps[:], onest, pt0[:], start=True, stop=False)
        nc.tensor.matmul(mae_ps[:], onest[0:P1, :], pt1[:], start=False, stop=True)

        # ---- per-(slot, chunk-pair) quadratic forms ----
        # S[(b,j), q*CL+cl] = sum_m (Us^T R_b Ut)[m,j]^2 * icap[j,m]
        S = cst.tile([BT, NQ * CL], f32, tag="S")
        for cl in range(CL):
            ict = icw[cl][:, 0:N]
            wkt = icw[cl][:, N : N + BT]
            u0 = us0[:, cl * N : (cl + 1) * N]
            u1 = us1[:, cl * N : (cl + 1) * N]
            for p in range(NP):
                q0, q1 = 2 * p, 2 * p + 1
                zt = ps_z.tile([BT, 2 * N], f32, tag="zt")
                for hi, q in ((0, q0), (1, q1)):
                    cz = zt[:, hi * N : (hi + 1) * N]
                    nc.tensor.matmul(
                        cz, rs0[:, q * BT : (q + 1) * BT], u0,
                        start=True, stop=False,
                    )
                    nc.tensor.matmul(
                        cz, rs1[:, q * BT : (q + 1) * BT], u1,
                        start=False, stop=True,
                    )
                ztsb = ztp.tile([BT, 2 * N], f16, tag="ztsb")
                if (cl * NP + p) % 2 == 0:
                    nc.scalar.copy(ztsb[:], zt[:])
                else:
                    nc.vector.tensor_copy(ztsb[:], zt[:])

                yt = ps_y.tile([BT, 2 * N], f32, tag="yt")
                nc.tensor.matmul(yt[:], wkt, ztsb[:], start=True, stop=True)
                sq = sqp.tile([BT, 2 * N], f32, tag="sq")
                nc.scalar.activation(sq[:], yt[:], AF.Square)
                for hi, q in ((0, q0), (1, q1)):
                    scr = scp.tile([BT, N], f32, tag="scr")
                    # scr = (sq * 1.0) * ic ; S[:,col] = sum_m scr
                    nc.vector.scalar_tensor_tensor(
                        scr[:],
                        sq[:, hi * N : (hi + 1) * N],
                        1.0,
                        ict,
                        op0=OP.mult,
                        op1=OP.mult,
                        accum_out=S[:, q * CL + cl : q * CL + cl + 1],
                    )

        # ---- quad[b, (q,cl)] = sum_j S[(b,j), (q,cl)] ----
        q_ps = ps_s.tile([BL, NQ * CL], f32, tag="q_ps")
        nc.tensor.matmul(q_ps[:], emt, S[:], start=True, stop=True)

        # ---- pack outputs: quad (8, 12) + [mae_abs, mae_cnt] on row 0 ----
        ot = cst.tile([BL, NQ * CL + 2], f32, tag="ot")
        nc.scalar.copy(ot[:, 0 : NQ * CL], q_ps[:])
        nc.vector.tensor_scalar(
            ot[:, NQ * CL : NQ * CL + 2], q_ps[:, 0:2], 0.0, None, op0=OP.mult
        )
        nc.scalar.copy(ot[0:1, NQ * CL : NQ * CL + 2], mae_ps[:])
        nc.sync.dma_start(io["oq"][:], ot[:])


def _build_program():
    import concourse.bacc as bacc
    import concourse.mybir as mybir
    from concourse import tile

    f32 = mybir.dt.float32
    nc = bacc.Bacc(None, target_bir_lowering=False)
    io = _declare_io(nc, f32)
    with tile.TileContext(nc) as tc:
        _emit_body(nc, tc, io)
    nc.compile()
    return nc


def _get_program():
    if "nc" not in _CACHE:
        _CACHE["nc"] = _build_program()
    return _CACHE["nc"]


def _host_prep(mu, target, unscaled_target, w, sigma, L_spatial, L_temporal):
    """Builds per-core input maps and the host-side ll constants."""
    f = np.float32
    h = np.float16
    mu = np.asarray(mu, dtype=f)
    target = np.asarray(target, dtype=f)
    unscaled_target = np.asarray(unscaled_target, dtype=f)
    Ls = np.asarray(L_spatial, dtype=np.float64)
    Lt = np.asarray(L_temporal, dtype=np.float64)

    Ks = Ls @ np.transpose(Ls, (0, 2, 1))
    Kt = Lt @ np.transpose(Lt, (0, 2, 1))
    Ds, Us = np.linalg.eigh(Ks)                   # (C, N), (C, N, N)
    Dt, Ut = np.linalg.eigh(Kt)                   # (C, T), (C, T, T)
    sig2 = np.asarray(sigma, dtype=np.float64) ** 2
    icap = 1.0 / (Dt[:, :, None] * Ds[:, None, :] + sig2[:, None, None])

    resid = (target - mu).transpose(1, 0, 2).reshape(N, B * T)      # n, (b,t)
    masku = (unscaled_target != 0).astype(np.uint8)
    masku = masku.transpose(1, 0, 2).reshape(N, B * T)

    em = np.kron(np.eye(BL, dtype=f), np.ones((T, 1), dtype=f))     # (96, 8)
    Us16 = Us.astype(h)
    ic16 = np.tile(icap, (1, BL, 1)).astype(h)                       # (C, 96, N)
    wk16 = np.stack([np.kron(np.eye(BL), Ut[c]) for c in range(C)]).astype(h)

    Ulog = np.sum(np.log(np.einsum("cii->ci", Ls)), axis=1)
    Vlog = np.sum(np.log(np.einsum("cii->ci", Lt)), axis=1)
    logw = np.log(np.asarray(w, dtype=np.float64)[..., 0])
    m2_full = (
        -0.5 * NT * LOG2PI + N * Vlog[None, :] + T * Ulog[None, :] + logw
    ).astype(f)                                                      # (B, C)

    aux = np.zeros((P0, BL + 1), dtype=f)
    aux[0:BT, 0:BL] = em
    aux[:, BL] = 1.0

    in_maps = []
    for k in range(NCORES):
        g, hh = k // G_B, k % G_B
        comps = COMP_GROUPS[g]
        bsl = slice(hh * BTL, (hh + 1) * BTL)

        d16a = np.zeros((N, D16A_W), dtype=h)
        d16a[:, 0:BTL] = resid[:, bsl].astype(h)
        d16a[:, MK_OFF:US_OFF] = (
            np.ascontiguousarray(masku[:, bsl]).view(h)
        )
        for cl, c in enumerate(comps):
            d16a[:, US_OFF + cl * N : US_OFF + (cl + 1) * N] = Us16[c]
        d16b = np.zeros((BT, D16B_W), dtype=h)
        for cl, c in enumerate(comps):
            d16b[:, cl * SLOT_W : cl * SLOT_W + N] = ic16[c]
            d16b[:, cl * SLOT_W + N : (cl + 1) * SLOT_W] = wk16[c]

        in_maps.append({"d16a": d16a, "d16b": d16b, "aux": aux})
    return in_maps, m2_full


def _host_final(results, m2_full):
    quad = np.zeros((B, C), dtype=np.float32)
    for k in range(NCORES):
        g, h = k // G_B, k % G_B
        comps = COMP_GROUPS[g]
        oq = results[k]["oq"]
        for cl, c in enumerate(comps):
            for q in range(NQ):
                b0 = h * BH + q * BL
                quad[b0 : b0 + BL, c] = oq[:, q * CL + cl]
    sum_abs = float(results[0]["oq"][0, NQ * CL]) + float(
        results[1]["oq"][0, NQ * CL]
    )
    sum_msk = float(results[0]["oq"][0, NQ * CL + 1]) + float(
        results[1]["oq"][0, NQ * CL + 1]
    )

    ll = m2_full - np.float32(0.5) * quad
    mx = ll.max(axis=1, keepdims=True)
    lse = np.log(np.exp(ll - mx).sum(axis=1, keepdims=True, dtype=np.float32)) + mx
    nll_loss = -np.float32(lse.sum()) / np.float32(B)
    mse_loss = np.float32(sum_abs) / np.float32(sum_msk)
    out = np.float32(RHO) * nll_loss + np.float32(1.0 - RHO) * mse_loss
    return np.asarray(out, dtype=np.float32)


def kernel(**inputs) -> np.ndarray:
    from concourse.bass_utils import run_bass_kernel_spmd

    nc = _get_program()
    in_maps, m2_full = _host_prep(
        inputs["mu"],
        inputs["target"],
        inputs["unscaled_target"],
        inputs["w"],
        inputs["sigma"],
        inputs["L_spatial"],
        inputs["L_temporal"],
    )
    res = run_bass_kernel_spmd(nc, in_maps, list(range(NCORES))).results
    return _host_final(res, m2_full)
